# revision 56
# baseline (speedup 1.0000x reference)
"""Trainium2 Bass kernel for MultiHeadAttention + residual + LayerNorm.

Sharding: 8 cores = 4 batches x 2 query-halves. Each core computes, for its
(batch b, half q): K/V projections for the whole batch (2048 tokens, all 16
heads), Q projection for its 1024 query tokens, full attention for those
queries, the complete output projection, residual add and LayerNorm for its
token slice. Zero inter-core communication; the host concatenates the 8
[1024, 1024] slices.

Optimizations vs the 531us fp16 baseline (measured ~395us, rel err 3.5e-3):
  - Q/K/V/out projections run in fp8-e4m3 with perf_mode=DoubleRow: the PE
    contracts 256 model dims per 512-col pass (2x the fp16 rate; measured
    216ns/MM at K=256). Weights are scaled x8 on the host to sit in fp8's
    normal range; the PSUM evacuation rescales.
  - K^T, Q^T, V stay SBUF-resident (no DRAM round-trips). V is laid out per
    head as [tok, 128] with cols 0..63 = 1/16 so the attn@V matmul produces
    the softmax denominator (pre-scaled for the x16 fp8 ctx quantization)
    for free in PSUM rows 0..63.
  - The softmax exp (the former single-engine bottleneck: 33.6M exps/core)
    is split across two engines: ~10/16 kv-chunks per slot use the Scalar
    engine's Exp activation; ~6/16 use a Schraudolph-style bit-trick on the
    Vector engine (i16 = s*1024*log2(e) + bias, bitcast to f16), max rel
    err ~3% which largely cancels in the softmax ratio.
  - Q/K projection evacuations run on the Scalar engine (activation
    Identity with per-partition scale+bias), keeping the Vector engine free
    for the exp work; V evacuation is a single Vector scalar_tensor_tensor.
  - Each attention slot's softmax reciprocal+multiply is emitted one slot
    later ("deferred finalize") so it never blocks the next slot's exps in
    the engine FIFOs (head-of-line avoidance).
  - Residual + output bias enter the out-proj PSUM via a scaled identity
    matmul (PSUM holds 128*y; LayerNorm is scale-invariant up to eps). LN
    uses bn_stats/bn_aggr on Vector, normalize on Scalar (scale=1/std,
    bias=-mean/std), gamma on Vector, beta on GpSimd.
  - Dummy warm-up matmuls keep the PE HAM clock gate open during the
    initial input DMA wait.
"""

import os
import sys

import numpy as np

for _p in ("/opt/trn_rl_repo", "/root/.axon_site/_ro/trn_rl_repo"):
    if os.path.isdir(_p) and _p not in sys.path:
        sys.path.insert(0, _p)

P = 128          # partitions
D = 1024         # model dim
C4 = 4           # 256-wide contraction chunks of the model dim
J2 = 2           # DoubleRow k-interleave
SQ = 1024        # query tokens per core
T = 2048         # kv tokens per core (one batch)
H = 16           # heads
HP = 8           # head pairs
DK = 64          # head dim
NT = 512         # matmul free-dim tile
N_CORES = 8
B, S = 4, 2048   # full problem

N_WARMUP_MM = 52

# Schraudolph exp in f16: i16 = trunc(s * 1024/ln2 + (15 - c)*1024 + 0.5)
EXPA = 1024.0 / float(np.log(2.0))
EXPB = (15.0 - 0.0434) * 1024.0 + 0.5

_CACHE = {}
LAST_RESULTS = None


def _emit(tc, t):
    import concourse.bass as bass  # noqa: F401
    from concourse import mybir
    from contextlib import ExitStack

    nc = tc.nc
    f32 = mybir.dt.float32
    f16 = mybir.dt.float16
    i16 = mybir.dt.int16
    f8 = mybir.dt.float8e4
    AF = mybir.ActivationFunctionType
    OP = mybir.AluOpType
    DR = mybir.MatmulPerfMode.DoubleRow

    xt8, xtq8, wq8, wk8, wv8, wo8 = (
        t["xt8"], t["xtq8"], t["wq8"], t["wk8"], t["wv8"], t["wo8"])
    xqbo, bq8, bk, cgb, ident, out = (
        t["xqbo"], t["bq8"], t["bk"], t["cgb"], t["ident"], t["out"])

    with ExitStack() as top:
        persist = top.enter_context(tc.tile_pool(name="persist", bufs=1))
        # broadcast constants: rows of [bv | gamma | beta], each [128, 1024]
        cbc = persist.tile([P, 3 * D], f16, tag="cbc")
        kt_sb = persist.tile([P, HP, T], f16, tag="kt")       # K^T resident
        qt_sb = persist.tile([P, HP, SQ], f16, tag="qt")      # Q^T resident
        # V resident: [tok%128, tokchunk, head, 128] with cols 0:64 = ones
        v_sb = persist.tile([P, T // P, H, P], f16, tag="v")
        ctxt8 = persist.tile([P, C4, J2, SQ], f8, tag="ctxt")  # ctx^T fp8 x16
        eps_t = persist.tile([P, 1], f32, tag="eps")
        id_sb = persist.tile([P, P], f16, tag="ident")
        bq_sb = persist.tile([P, HP], f32, tag="bq")
        bk_sb = persist.tile([P, HP], f32, tag="bk")

        nc.vector.memset(eps_t[:], 1e-5)
        # "ones" block of V (cols 0:64 of every [tok,128] head tile): 1/16 so
        # the attn@V denominator comes out pre-divided for the x16 fp8 ctx
        # scale (ctx*16 = num / (den/16)). On GpSimd so it doesn't delay the
        # Vector-queued warm-up operand memsets below.
        nc.gpsimd.memset(v_sb[:, :, :, 0:DK], 1.0 / 16.0)
        nc.sync.dma_start(id_sb[:], ident[:])
        nc.sync.dma_start(bq_sb[:], bq8[:].rearrange("(dc p) -> p dc", p=P))
        nc.sync.dma_start(bk_sb[:], bk[:].rearrange("(dc p) -> p dc", p=P))

        ones1 = persist.tile([1, P], f32, tag="ones1")
        csrow = persist.tile([1, 3 * D], f32, tag="csrow")
        wmz = persist.tile([P, NT], f16, tag="wmz")
        nc.vector.memset(wmz[:], 0.0)
        nc.vector.memset(ones1[:], 1.0)
        nc.sync.dma_start(csrow[:], cgb[:].rearrange("(o n) -> o n", o=1))

        # ---- warm-up + constant broadcast (PE busy during input DMA wait) ----
        with tc.tile_pool(name="wu_psum", bufs=2, space="PSUM") as wps:
            for i in range(N_WARMUP_MM):
                wp = wps.tile([P, NT], f32, tag="wu")
                nc.tensor.matmul(wp[:], lhsT=wmz[:, 0:P], rhs=wmz[:],
                                 start=True, stop=True)
            for i in range(6):
                pt_ = wps.tile([P, NT], f32, tag="wu")
                nc.tensor.matmul(pt_[:], lhsT=ones1[:],
                                 rhs=csrow[:, i * NT:(i + 1) * NT],
                                 start=True, stop=True)
                nc.vector.tensor_copy(cbc[:, i * NT:(i + 1) * NT], pt_[:])

        # ---------------- Phase 1: Q/K/V projections (fp8 DoubleRow) ----------------
        with ExitStack() as p1:
            wp1 = p1.enter_context(tc.tile_pool(name="wqkv", bufs=1))
            wq_sb = wp1.tile([P, C4, J2, D], f8, tag="wq")
            wk_sb = wp1.tile([P, C4, J2, D], f8, tag="wk")
            wv_sb = wp1.tile([P, C4, J2, D], f8, tag="wv")
            xt_sb = wp1.tile([P, C4, J2, T], f8, tag="xt")
            xtq_sb = wp1.tile([P, C4, J2, SQ], f8, tag="xtq")
            # DMA priority order: Q's operands first so the PE can start
            # early; xtq/wq split per 256-row chunk so the first Q matmul's
            # dependencies resolve after ~a quarter of the transfer.
            for c in range(C4):
                nc.sync.dma_start(
                    xtq_sb[:, c], xtq8[c * 2 * P:(c + 1) * 2 * P, :].rearrange(
                        "(j p) s -> p j s", p=P))
                nc.sync.dma_start(
                    wq_sb[:, c], wq8[c * 2 * P:(c + 1) * 2 * P, :].rearrange(
                        "(j p) d -> p j d", p=P))
            nc.sync.dma_start(wk_sb[:], wk8[:].rearrange("(c j p) d -> p c j d", p=P, j=J2))
            nc.sync.dma_start(xt_sb[:], xt8[:].rearrange("(c j p) s -> p c j s", p=P, j=J2))
            nc.sync.dma_start(wv_sb[:], wv8[:].rearrange("(c j p) d -> p c j d", p=P, j=J2))
            pv = p1.enter_context(tc.tile_pool(name="ps1v", bufs=4, space="PSUM"))
            pp = p1.enter_context(tc.tile_pool(name="ps1", bufs=4, space="PSUM"))

            # Q: stationary weight chunk reused across both query tiles.
            for dc in range(HP):
                for st in range(SQ // NT):
                    ps = pp.tile([P, NT], f32, tag="ps")
                    for c in range(C4):
                        nc.tensor.matmul(
                            ps[:], lhsT=wq_sb[:, c, :, dc * P:(dc + 1) * P],
                            rhs=xtq_sb[:, c, :, st * NT:(st + 1) * NT],
                            start=(c == 0), stop=(c == C4 - 1), perf_mode=DR)
                    # Q^T = psum/64 + bq/8  (scores pre-scaled by 1/sqrt(dk))
                    nc.scalar.activation(qt_sb[:, dc, st * NT:(st + 1) * NT],
                                         ps[:], AF.Identity,
                                         bias=bq_sb[:, dc:dc + 1], scale=1.0 / 64.0)

            # K: stationary weight chunk reused across four kv tiles.
            for dc in range(HP):
                for tt in range(T // NT):
                    ps = pp.tile([P, NT], f32, tag="ps")
                    for c in range(C4):
                        nc.tensor.matmul(
                            ps[:], lhsT=wk_sb[:, c, :, dc * P:(dc + 1) * P],
                            rhs=xt_sb[:, c, :, tt * NT:(tt + 1) * NT],
                            start=(c == 0), stop=(c == C4 - 1), perf_mode=DR)
                    nc.scalar.activation(kt_sb[:, dc, tt * NT:(tt + 1) * NT],
                                         ps[:], AF.Identity,
                                         bias=bk_sb[:, dc:dc + 1], scale=1.0 / 8.0)

            # V: stationary x chunk reused across both output-dim tiles.
            for tcg in range(T // P):
                pss = [pv.tile([P, NT], f32, tag="psv", name=f"psv{dt}")
                       for dt in range(2)]
                for c in range(C4):
                    for dt in range(2):
                        nc.tensor.matmul(
                            pss[dt][:], lhsT=xt_sb[:, c, :, tcg * P:(tcg + 1) * P],
                            rhs=wv_sb[:, c, :, dt * NT:(dt + 1) * NT],
                            start=(c == 0), stop=(c == C4 - 1), perf_mode=DR)
                for dt in range(2):
                    # V = psum/8 + bv, written per head into cols 64:128
                    nc.vector.scalar_tensor_tensor(
                        v_sb[:, tcg, dt * 8:(dt + 1) * 8, DK:P],
                        in0=pss[dt][:].rearrange("p (h k) -> p h k", k=DK),
                        scalar=1.0 / 8.0,
                        in1=cbc[:, dt * NT:(dt + 1) * NT].rearrange(
                            "p (h k) -> p h k", k=DK),
                        op0=OP.mult, op1=OP.add)

        # ---------------- Phases 2+3 scope: out-proj weight loads early ----------------
        p23 = top.enter_context(ExitStack())
        wp3 = p23.enter_context(tc.tile_pool(name="wo_pool", bufs=1))
        wo_sb = wp3.tile([P, C4, J2, D], f8, tag="wo")
        nc.sync.dma_start(wo_sb[:], wo8[:].rearrange("(c j p) d -> p c j d", p=P, j=J2))

        # ---------------- Phase 2: attention ----------------
        with ExitStack() as p2:
            ptp = p2.enter_context(tc.tile_pool(name="ptp", bufs=18))
            ccp = p2.enter_context(tc.tile_pool(name="ccp", bufs=4))
            sps = p2.enter_context(tc.tile_pool(name="sps", bufs=2, space="PSUM"))
            cps = p2.enter_context(tc.tile_pool(name="cps", bufs=4, space="PSUM"))

            def attn_slot(hp, st):
                """Scores + exp + attn@V for one (head-pair, query-tile) slot.
                Returns the finalize closure (PSUM evac + softmax divide),
                which the caller emits one slot later: that keeps the evac
                out of the exp engines' FIFO critical path."""
                c0 = cps.tile([P, NT], f32, tag="cps", name=f"c0_{hp}_{st}")
                c1 = cps.tile([P, NT], f32, tag="cps", name=f"c1_{hp}_{st}")
                pts = []

                def scores_exp(tcc):
                    sp = sps.tile([P, 2 * NT], f32, tag="sps", name=f"sp{tcc}")
                    nc.tensor.matmul(sp[:, 0:NT],
                                     lhsT=kt_sb[0:DK, hp, tcc * P:(tcc + 1) * P],
                                     rhs=qt_sb[0:DK, hp, st * NT:(st + 1) * NT],
                                     start=True, stop=True)
                    nc.tensor.matmul(sp[:, NT:2 * NT],
                                     lhsT=kt_sb[DK:P, hp, tcc * P:(tcc + 1) * P],
                                     rhs=qt_sb[DK:P, hp, st * NT:(st + 1) * NT],
                                     start=True, stop=True)
                    pt = ptp.tile([P, 2 * NT], f16, tag="pt", name=f"pt{tcc}")
                    # Exp split across Scalar(Exp table) and Vector(bit-trick
                    # exp); Vector also runs the softmax reciprocal+multiply,
                    # so Scalar takes 10/9 tiles and Vector 6/7.
                    if tcc % 2 == 0 or tcc == 13 or (tcc == 15 and st == 0):
                        nc.scalar.activation(pt[:], sp[:], AF.Exp)
                    else:
                        nc.vector.tensor_scalar(
                            out=pt[:].bitcast(i16), in0=sp[:],
                            scalar1=EXPA, scalar2=EXPB,
                            op0=OP.mult, op1=OP.add)
                    pts.append(pt)

                def attnv(tcc):
                    pt = pts[tcc]
                    nc.tensor.matmul(c0[:], lhsT=v_sb[:, tcc, 2 * hp, :],
                                     rhs=pt[:, 0:NT],
                                     start=(tcc == 0), stop=(tcc == T // P - 1))
                    nc.tensor.matmul(c1[:], lhsT=v_sb[:, tcc, 2 * hp + 1, :],
                                     rhs=pt[:, NT:2 * NT],
                                     start=(tcc == 0), stop=(tcc == T // P - 1))

                for tcc in range(T // P):
                    scores_exp(tcc)
                for tcc in range(T // P):
                    attnv(tcc)

                def finalize():
                    # ctx^T * 16 in fp8: num * (16/den), via reciprocal of
                    # the pre-scaled denominator (den/16 from the V block).
                    for h2, cc in ((0, c0), (1, c1)):
                        rec = ccp.tile([DK, NT], f32, tag="rec", name=f"rec{h2}")
                        nc.vector.reciprocal_approx_fast(rec[:], cc[0:DK, :])
                        nc.vector.tensor_tensor(
                            ctxt8[h2 * DK:(h2 + 1) * DK, hp // 2, hp % 2,
                                  st * NT:(st + 1) * NT],
                            cc[DK:P, :], rec[:], OP.mult)
                return finalize

            pending = None
            for hp in range(HP):
                for st in range(SQ // NT):
                    last = (hp == HP - 1 and st == SQ // NT - 1)
                    fin = attn_slot(hp, st)
                    if pending is not None:
                        pending()
                    if last:
                        # emit immediately: phase 3 waits on this slot's ctx
                        fin()
                        pending = None
                    else:
                        pending = fin

        # ---------------- Phase 3: out proj + residual + LayerNorm ----------------
        with ExitStack() as p3:
            stp = p3.enter_context(tc.tile_pool(name="stats", bufs=12))
            np_ = p3.enter_context(tc.tile_pool(name="norm", bufs=6))
            outp = p3.enter_context(tc.tile_pool(name="outp", bufs=6))
            xqp = p3.enter_context(tc.tile_pool(name="xqp", bufs=4))
            ops = p3.enter_context(tc.tile_pool(name="ps3", bufs=8, space="PSUM"))

            for sc in range(SQ // P):
                xqt = xqp.tile([P, D], f16, tag="xq")
                nc.sync.dma_start(xqt[:], xqbo[sc * P:(sc + 1) * P, :])
                pse = [ops.tile([P, NT], f32, tag="ps", name=f"ps3_{et}")
                       for et in range(2)]
                for c in range(C4):
                    for et in range(2):
                        nc.tensor.matmul(
                            pse[et][:], lhsT=ctxt8[:, c, :, sc * P:(sc + 1) * P],
                            rhs=wo_sb[:, c, :, et * NT:(et + 1) * NT],
                            start=(c == 0), stop=False, perf_mode=DR)
                # += 128*(x + bo) via scaled identity; PSUM = 128*y, and
                # LayerNorm is scale-invariant.
                for et in range(2):
                    nc.tensor.matmul(pse[et][:], lhsT=id_sb[:],
                                     rhs=xqt[:, et * NT:(et + 1) * NT],
                                     start=False, stop=True)
                bst = stp.tile([P, 2, 6], f32, tag="bst")
                for et in range(2):
                    nc.vector.bn_stats(bst[:, et, :], pse[et][:])
                mv = stp.tile([P, 2], f32, tag="mv")
                nc.vector.bn_aggr(mv[:], bst[:])
                std = stp.tile([P, 1], f32, tag="std")
                nc.scalar.activation(std[:], mv[:, 1:2], AF.Sqrt, bias=eps_t[:])
                rstd = stp.tile([P, 1], f32, tag="rstd")
                nc.vector.reciprocal(rstd[:], std[:])
                nmr = stp.tile([P, 1], f32, tag="nmr")
                nc.vector.scalar_tensor_tensor(nmr[:], in0=mv[:, 0:1], scalar=-1.0,
                                               in1=rstd[:], op0=OP.mult, op1=OP.mult)
                for et in range(2):
                    if t["gb_trivial"]:
                        # gamma==1, beta==0 (checked on the host): the
                        # normalize activation writes the final output —
                        # drops two engine hops from the drain chain.
                        o = outp.tile([P, NT], f32, tag="o")
                        nc.scalar.activation(o[:], pse[et][:], AF.Identity,
                                             bias=nmr[:], scale=rstd[:])
                    else:
                        tn = np_.tile([P, NT], f32, tag="tn")
                        nc.scalar.activation(tn[:], pse[et][:], AF.Identity,
                                             bias=nmr[:], scale=rstd[:])
                        o = outp.tile([P, NT], f32, tag="o")
                        nc.vector.tensor_tensor(o[:], tn[:],
                                                cbc[:, D + et * NT:D + (et + 1) * NT],
                                                OP.mult)
                        nc.gpsimd.tensor_tensor(o[:], o[:],
                                                cbc[:, 2 * D + et * NT:2 * D + (et + 1) * NT],
                                                OP.add)
                    nc.sync.dma_start(out[sc * P:(sc + 1) * P, et * NT:(et + 1) * NT],
                                      o[:])


def _build(gb_trivial):
    key = ("nc", gb_trivial)
    if key in _CACHE:
        return _CACHE[key]
    from concourse import bacc, mybir
    import concourse.tile as tile

    f32 = mybir.dt.float32
    f16 = mybir.dt.float16
    f8 = mybir.dt.float8e4
    nc = bacc.Bacc("TRN2", target_bir_lowering=False, debug=False)
    t = {}
    t["xt8"] = nc.dram_tensor("xt8", [D, T], f8, kind="ExternalInput")
    t["xtq8"] = nc.dram_tensor("xtq8", [D, SQ], f8, kind="ExternalInput")
    t["wq8"] = nc.dram_tensor("wq8", [D, D], f8, kind="ExternalInput")
    t["wk8"] = nc.dram_tensor("wk8", [D, D], f8, kind="ExternalInput")
    t["wv8"] = nc.dram_tensor("wv8", [D, D], f8, kind="ExternalInput")
    t["wo8"] = nc.dram_tensor("wo8", [D, D], f8, kind="ExternalInput")
    t["xqbo"] = nc.dram_tensor("xqbo", [SQ, D], f16, kind="ExternalInput")
    t["bq8"] = nc.dram_tensor("bq8", [D], f32, kind="ExternalInput")
    t["bk"] = nc.dram_tensor("bk", [D], f32, kind="ExternalInput")
    t["cgb"] = nc.dram_tensor("cgb", [3 * D], f32, kind="ExternalInput")
    t["ident"] = nc.dram_tensor("ident", [P, P], f16, kind="ExternalInput")
    t["out"] = nc.dram_tensor("out", [SQ, D], f32, kind="ExternalOutput")

    t["gb_trivial"] = gb_trivial
    with tile.TileContext(nc) as tc:
        _emit(tc, t)
    nc.compile()
    _CACHE[key] = nc
    return nc


def _chunk_fp8(a):
    """[1024, N] -> fp8 with rows pre-arranged as (c, j, p); layout is
    identity because d = c*256 + j*128 + p matches C-order reshape."""
    import ml_dtypes
    return np.clip(a, -240.0, 240.0).astype(ml_dtypes.float8_e4m3)


def _prep_inputs(x, Wq, bq, Wk, bk, Wv, bv, Wo, bo, ln_gamma, ln_beta):
    """Host-side sharding/layout prep. Returns per-core input maps."""
    f = np.float32
    x = np.asarray(x, f)
    wq8 = _chunk_fp8(np.asarray(Wq, f).T * 8.0)
    wk8 = _chunk_fp8(np.asarray(Wk, f).T * 8.0)
    wv8 = _chunk_fp8(np.asarray(Wv, f).T * 8.0)
    wo8 = _chunk_fp8(np.asarray(Wo, f).T * 8.0)
    bq8 = np.asarray(bq, f) / 8.0
    cgb = np.concatenate([np.asarray(bv, f), np.asarray(ln_gamma, f),
                          np.asarray(ln_beta, f)])
    ident = (128.0 * np.eye(P, dtype=f)).astype(np.float16)
    bo_f = np.asarray(bo, f)
    in_maps = []
    for c in range(N_CORES):
        b, half = c // 2, c % 2
        xb = x[b]                                        # [2048, 1024]
        xslice = xb[half * SQ:(half + 1) * SQ]           # [1024, 1024]
        in_maps.append({
            "xt8": _chunk_fp8(np.ascontiguousarray(xb.T)),
            "xtq8": _chunk_fp8(np.ascontiguousarray(xslice.T)),
            "wq8": wq8, "wk8": wk8, "wv8": wv8, "wo8": wo8,
            "xqbo": (xslice + bo_f).astype(np.float16),
            "bq8": bq8, "bk": np.asarray(bk, f),
            "cgb": cgb, "ident": ident,
        })
    return in_maps


def _ensure_axon_hooks_shim():
    """This image's `antenv` lacks the `axon_hooks` registry module that
    `run_bass_kernel_spmd(trace=True)` imports. Provide it (hook installed
    from the boot .so when available, else None -> tracing degrades
    gracefully instead of raising ImportError)."""
    import importlib
    import types

    try:
        importlib.import_module("antenv.axon_hooks")
        return
    except ImportError:
        pass
    mod = types.ModuleType("antenv.axon_hooks")
    _state = {"hook": None}
    mod.set_axon_ntff_profile_hook = lambda h: _state.update(hook=h)
    mod.get_axon_ntff_profile_hook = lambda: _state["hook"]
    sys.modules["antenv.axon_hooks"] = mod
    try:
        import antenv
        antenv.axon_hooks = mod
    except Exception:
        pass
    try:
        from trn_agent_boot.trn_boot import _ntff_profile_via_ctypes
        so = "/opt/axon/libaxon_pjrt.so"
        if os.path.exists(so):
            mod.set_axon_ntff_profile_hook(_ntff_profile_via_ctypes(so))
    except Exception:
        pass


def kernel(**inputs):
    global LAST_RESULTS
    _ensure_axon_hooks_shim()
    from concourse.bass_utils import run_bass_kernel_spmd

    gb_trivial = (np.allclose(np.asarray(inputs["ln_gamma"]), 1.0)
                  and np.allclose(np.asarray(inputs["ln_beta"]), 0.0))
    nc = _build(gb_trivial)
    in_maps = _prep_inputs(**inputs)
    trace = bool(os.environ.get("MHA_TRACE"))
    res = run_bass_kernel_spmd(nc, in_maps, core_ids=list(range(N_CORES)),
                               trace=trace)
    LAST_RESULTS = res
    out = np.empty((B, S, D), np.float32)
    for c in range(N_CORES):
        b, half = c // 2, c % 2
        out[b, half * SQ:(half + 1) * SQ, :] = res.results[c]["out"]
    return out


if __name__ == "__main__":
    from reference import setup_inputs, reference
    import jax
    with jax.default_device(jax.devices("cpu")[0]):
        inp = {k: np.asarray(v) for k, v in setup_inputs().items()}
        exp = np.asarray(reference(**inp))
    act = kernel(**inp)
    err = np.linalg.norm(act - exp) / np.linalg.norm(exp)
    print("Relative error:", err)


# revision 57
# speedup vs baseline: 1.0082x; 1.0082x over previous
"""Trainium2 Bass kernel for MultiHeadAttention + residual + LayerNorm.

Sharding: 8 cores = 4 batches x 2 query-halves. Each core computes, for its
(batch b, half q): K/V projections for the whole batch (2048 tokens, all 16
heads), Q projection for its 1024 query tokens, full attention for those
queries, the complete output projection, residual add and LayerNorm for its
token slice. Zero inter-core communication; the host concatenates the 8
[1024, 1024] slices.

Optimizations vs the 531us fp16 baseline (measured ~395us, rel err 3.5e-3):
  - Q/K/V/out projections run in fp8-e4m3 with perf_mode=DoubleRow: the PE
    contracts 256 model dims per 512-col pass (2x the fp16 rate; measured
    216ns/MM at K=256). Weights are scaled x8 on the host to sit in fp8's
    normal range; the PSUM evacuation rescales.
  - K^T, Q^T, V stay SBUF-resident (no DRAM round-trips). V is laid out per
    head as [tok, 128] with cols 0..63 = 1/16 so the attn@V matmul produces
    the softmax denominator (pre-scaled for the x16 fp8 ctx quantization)
    for free in PSUM rows 0..63.
  - The softmax exp (the former single-engine bottleneck: 33.6M exps/core)
    is split across two engines: ~10/16 kv-chunks per slot use the Scalar
    engine's Exp activation; ~6/16 use a Schraudolph-style bit-trick on the
    Vector engine (i16 = s*1024*log2(e) + bias, bitcast to f16), max rel
    err ~3% which largely cancels in the softmax ratio.
  - Q/K projection evacuations run on the Scalar engine (activation
    Identity with per-partition scale+bias), keeping the Vector engine free
    for the exp work; V evacuation is a single Vector scalar_tensor_tensor.
  - Each attention slot's softmax reciprocal+multiply is emitted one slot
    later ("deferred finalize") so it never blocks the next slot's exps in
    the engine FIFOs (head-of-line avoidance).
  - Residual + output bias enter the out-proj PSUM via a scaled identity
    matmul (PSUM holds 128*y; LayerNorm is scale-invariant up to eps). LN
    uses bn_stats/bn_aggr on Vector, normalize on Scalar (scale=1/std,
    bias=-mean/std), gamma on Vector, beta on GpSimd.
  - Dummy warm-up matmuls keep the PE HAM clock gate open during the
    initial input DMA wait.
"""

import os
import sys

import numpy as np

for _p in ("/opt/trn_rl_repo", "/root/.axon_site/_ro/trn_rl_repo"):
    if os.path.isdir(_p) and _p not in sys.path:
        sys.path.insert(0, _p)

P = 128          # partitions
D = 1024         # model dim
C4 = 4           # 256-wide contraction chunks of the model dim
J2 = 2           # DoubleRow k-interleave
SQ = 1024        # query tokens per core
T = 2048         # kv tokens per core (one batch)
H = 16           # heads
HP = 8           # head pairs
DK = 64          # head dim
NT = 512         # matmul free-dim tile
N_CORES = 8
B, S = 4, 2048   # full problem

N_WARMUP_MM = 52

# Schraudolph exp in f16: i16 = trunc(s * 1024/ln2 + (15 - c)*1024 + 0.5)
EXPA = 1024.0 / float(np.log(2.0))
EXPB = (15.0 - 0.0434) * 1024.0 + 0.5

_CACHE = {}
LAST_RESULTS = None


def _emit(tc, t):
    import concourse.bass as bass  # noqa: F401
    from concourse import mybir
    from contextlib import ExitStack

    nc = tc.nc
    f32 = mybir.dt.float32
    f16 = mybir.dt.float16
    i16 = mybir.dt.int16
    f8 = mybir.dt.float8e4
    AF = mybir.ActivationFunctionType
    OP = mybir.AluOpType
    DR = mybir.MatmulPerfMode.DoubleRow

    xt8, xtq8, wq8, wk8, wv8, wo8 = (
        t["xt8"], t["xtq8"], t["wq8"], t["wk8"], t["wv8"], t["wo8"])
    xqbo, bq8, bk, cgb, ident, out = (
        t["xqbo"], t["bq8"], t["bk"], t["cgb"], t["ident"], t["out"])

    with ExitStack() as top:
        persist = top.enter_context(tc.tile_pool(name="persist", bufs=1))
        # broadcast constants: rows of [bv | gamma | beta], each [128, 1024]
        cbc = persist.tile([P, 3 * D], f16, tag="cbc")
        kt_sb = persist.tile([P, HP, T], f16, tag="kt")       # K^T resident
        qt_sb = persist.tile([P, HP, SQ], f16, tag="qt")      # Q^T resident
        # V resident: [tok%128, tokchunk, head, 128] with cols 0:64 = ones
        v_sb = persist.tile([P, T // P, H, P], f16, tag="v")
        ctxt8 = persist.tile([P, C4, J2, SQ], f8, tag="ctxt")  # ctx^T fp8 x16
        eps_t = persist.tile([P, 1], f32, tag="eps")
        id_sb = persist.tile([P, P], f16, tag="ident")
        bq_sb = persist.tile([P, HP], f32, tag="bq")
        bk_sb = persist.tile([P, HP], f32, tag="bk")

        nc.vector.memset(eps_t[:], 1e-5)
        # "ones" block of V (cols 0:64 of every [tok,128] head tile): 1/16 so
        # the attn@V denominator comes out pre-divided for the x16 fp8 ctx
        # scale (ctx*16 = num / (den/16)). On GpSimd so it doesn't delay the
        # Vector-queued warm-up operand memsets below.
        nc.gpsimd.memset(v_sb[:, :, :, 0:DK], 1.0 / 16.0)
        nc.sync.dma_start(id_sb[:], ident[:])
        nc.sync.dma_start(bq_sb[:], bq8[:].rearrange("(dc p) -> p dc", p=P))
        nc.sync.dma_start(bk_sb[:], bk[:].rearrange("(dc p) -> p dc", p=P))

        ones1 = persist.tile([1, P], f32, tag="ones1")
        csrow = persist.tile([1, 3 * D], f32, tag="csrow")
        wmz = persist.tile([P, NT], f16, tag="wmz")
        nc.vector.memset(wmz[:], 0.0)
        nc.vector.memset(ones1[:], 1.0)
        nc.sync.dma_start(csrow[:], cgb[:].rearrange("(o n) -> o n", o=1))

        # ---- warm-up + constant broadcast (PE busy during input DMA wait) ----
        with tc.tile_pool(name="wu_psum", bufs=2, space="PSUM") as wps:
            for i in range(N_WARMUP_MM):
                wp = wps.tile([P, NT], f32, tag="wu")
                nc.tensor.matmul(wp[:], lhsT=wmz[:, 0:P], rhs=wmz[:],
                                 start=True, stop=True)
            for i in range(6):
                pt_ = wps.tile([P, NT], f32, tag="wu")
                nc.tensor.matmul(pt_[:], lhsT=ones1[:],
                                 rhs=csrow[:, i * NT:(i + 1) * NT],
                                 start=True, stop=True)
                nc.vector.tensor_copy(cbc[:, i * NT:(i + 1) * NT], pt_[:])

        # ---------------- Phase 1: Q/K/V projections (fp8 DoubleRow) ----------------
        with ExitStack() as p1:
            wp1 = p1.enter_context(tc.tile_pool(name="wqkv", bufs=1))
            wq_sb = wp1.tile([P, C4, J2, D], f8, tag="wq")
            wk_sb = wp1.tile([P, C4, J2, D], f8, tag="wk")
            wv_sb = wp1.tile([P, C4, J2, D], f8, tag="wv")
            xt_sb = wp1.tile([P, C4, J2, T], f8, tag="xt")
            xtq_sb = wp1.tile([P, C4, J2, SQ], f8, tag="xtq")
            # DMA priority order: Q's operands first so the PE can start
            # early; xtq/wq split per 256-row chunk so the first Q matmul's
            # dependencies resolve after ~a quarter of the transfer.
            for c in range(C4):
                nc.sync.dma_start(
                    xtq_sb[:, c], xtq8[c * 2 * P:(c + 1) * 2 * P, :].rearrange(
                        "(j p) s -> p j s", p=P))
                nc.sync.dma_start(
                    wq_sb[:, c], wq8[c * 2 * P:(c + 1) * 2 * P, :].rearrange(
                        "(j p) d -> p j d", p=P))
            nc.sync.dma_start(wk_sb[:], wk8[:].rearrange("(c j p) d -> p c j d", p=P, j=J2))
            nc.sync.dma_start(xt_sb[:], xt8[:].rearrange("(c j p) s -> p c j s", p=P, j=J2))
            nc.sync.dma_start(wv_sb[:], wv8[:].rearrange("(c j p) d -> p c j d", p=P, j=J2))
            pv = p1.enter_context(tc.tile_pool(name="ps1v", bufs=4, space="PSUM"))
            pp = p1.enter_context(tc.tile_pool(name="ps1", bufs=4, space="PSUM"))

            # Q: stationary weight chunk reused across both query tiles.
            for dc in range(HP):
                for st in range(SQ // NT):
                    ps = pp.tile([P, NT], f32, tag="ps")
                    for c in range(C4):
                        nc.tensor.matmul(
                            ps[:], lhsT=wq_sb[:, c, :, dc * P:(dc + 1) * P],
                            rhs=xtq_sb[:, c, :, st * NT:(st + 1) * NT],
                            start=(c == 0), stop=(c == C4 - 1), perf_mode=DR)
                    # Q^T = psum/64 + bq/8  (scores pre-scaled by 1/sqrt(dk))
                    nc.scalar.activation(qt_sb[:, dc, st * NT:(st + 1) * NT],
                                         ps[:], AF.Identity,
                                         bias=bq_sb[:, dc:dc + 1], scale=1.0 / 64.0)

            # K: stationary weight chunk reused across four kv tiles.
            for dc in range(HP):
                for tt in range(T // NT):
                    ps = pp.tile([P, NT], f32, tag="ps")
                    for c in range(C4):
                        nc.tensor.matmul(
                            ps[:], lhsT=wk_sb[:, c, :, dc * P:(dc + 1) * P],
                            rhs=xt_sb[:, c, :, tt * NT:(tt + 1) * NT],
                            start=(c == 0), stop=(c == C4 - 1), perf_mode=DR)
                    nc.scalar.activation(kt_sb[:, dc, tt * NT:(tt + 1) * NT],
                                         ps[:], AF.Identity,
                                         bias=bk_sb[:, dc:dc + 1], scale=1.0 / 8.0)

            # V: stationary x chunk reused across both output-dim tiles.
            for tcg in range(T // P):
                pss = [pv.tile([P, NT], f32, tag="psv", name=f"psv{dt}")
                       for dt in range(2)]
                for c in range(C4):
                    for dt in range(2):
                        nc.tensor.matmul(
                            pss[dt][:], lhsT=xt_sb[:, c, :, tcg * P:(tcg + 1) * P],
                            rhs=wv_sb[:, c, :, dt * NT:(dt + 1) * NT],
                            start=(c == 0), stop=(c == C4 - 1), perf_mode=DR)
                for dt in range(2):
                    # V = psum/8 + bv, written per head into cols 64:128
                    nc.vector.scalar_tensor_tensor(
                        v_sb[:, tcg, dt * 8:(dt + 1) * 8, DK:P],
                        in0=pss[dt][:].rearrange("p (h k) -> p h k", k=DK),
                        scalar=1.0 / 8.0,
                        in1=cbc[:, dt * NT:(dt + 1) * NT].rearrange(
                            "p (h k) -> p h k", k=DK),
                        op0=OP.mult, op1=OP.add)

        # ---------------- Phases 2+3 scope: out-proj weight loads early ----------------
        p23 = top.enter_context(ExitStack())
        wp3 = p23.enter_context(tc.tile_pool(name="wo_pool", bufs=1))
        wo_sb = wp3.tile([P, C4, J2, D], f8, tag="wo")
        nc.sync.dma_start(wo_sb[:], wo8[:].rearrange("(c j p) d -> p c j d", p=P, j=J2))

        # ---------------- Phase 2: attention ----------------
        with ExitStack() as p2:
            ptp = p2.enter_context(tc.tile_pool(name="ptp", bufs=18))
            ccp = p2.enter_context(tc.tile_pool(name="ccp", bufs=4))
            sps = p2.enter_context(tc.tile_pool(name="sps", bufs=2, space="PSUM"))
            cps = p2.enter_context(tc.tile_pool(name="cps", bufs=4, space="PSUM"))

            def attn_slot(hp, st):
                """Scores + exp + attn@V for one (head-pair, query-tile) slot.
                Returns the finalize closure (PSUM evac + softmax divide),
                which the caller emits one slot later: that keeps the evac
                out of the exp engines' FIFO critical path."""
                c0 = cps.tile([P, NT], f32, tag="cps", name=f"c0_{hp}_{st}")
                c1 = cps.tile([P, NT], f32, tag="cps", name=f"c1_{hp}_{st}")
                pts = []

                def scores_exp(tcc):
                    sp = sps.tile([P, 2 * NT], f32, tag="sps", name=f"sp{tcc}")
                    nc.tensor.matmul(sp[:, 0:NT],
                                     lhsT=kt_sb[0:DK, hp, tcc * P:(tcc + 1) * P],
                                     rhs=qt_sb[0:DK, hp, st * NT:(st + 1) * NT],
                                     start=True, stop=True)
                    nc.tensor.matmul(sp[:, NT:2 * NT],
                                     lhsT=kt_sb[DK:P, hp, tcc * P:(tcc + 1) * P],
                                     rhs=qt_sb[DK:P, hp, st * NT:(st + 1) * NT],
                                     start=True, stop=True)
                    pt = ptp.tile([P, 2 * NT], f16, tag="pt", name=f"pt{tcc}")
                    # Exp split across Scalar(Exp table) and Vector(bit-trick
                    # exp); Vector also runs the softmax reciprocal+multiply,
                    # so Scalar takes 10/9 tiles and Vector 6/7.
                    if tcc % 2 == 0 or tcc == 13:
                        nc.scalar.activation(pt[:], sp[:], AF.Exp)
                    else:
                        nc.vector.tensor_scalar(
                            out=pt[:].bitcast(i16), in0=sp[:],
                            scalar1=EXPA, scalar2=EXPB,
                            op0=OP.mult, op1=OP.add)
                    pts.append(pt)

                def attnv(tcc):
                    pt = pts[tcc]
                    nc.tensor.matmul(c0[:], lhsT=v_sb[:, tcc, 2 * hp, :],
                                     rhs=pt[:, 0:NT],
                                     start=(tcc == 0), stop=(tcc == T // P - 1))
                    nc.tensor.matmul(c1[:], lhsT=v_sb[:, tcc, 2 * hp + 1, :],
                                     rhs=pt[:, NT:2 * NT],
                                     start=(tcc == 0), stop=(tcc == T // P - 1))

                for tcc in range(T // P):
                    scores_exp(tcc)
                for tcc in range(T // P):
                    attnv(tcc)

                def finalize():
                    # ctx^T * 16 in fp8: num * (16/den), via reciprocal of
                    # the pre-scaled denominator (den/16 from the V block).
                    for h2, cc in ((0, c0), (1, c1)):
                        rec = ccp.tile([DK, NT], f32, tag="rec", name=f"rec{h2}")
                        nc.vector.reciprocal_approx_fast(rec[:], cc[0:DK, :])
                        nc.vector.tensor_tensor(
                            ctxt8[h2 * DK:(h2 + 1) * DK, hp // 2, hp % 2,
                                  st * NT:(st + 1) * NT],
                            cc[DK:P, :], rec[:], OP.mult)
                return finalize

            pending = None
            for hp in range(HP):
                for st in range(SQ // NT):
                    last = (hp == HP - 1 and st == SQ // NT - 1)
                    fin = attn_slot(hp, st)
                    if pending is not None:
                        pending()
                    if last:
                        # emit immediately: phase 3 waits on this slot's ctx
                        fin()
                        pending = None
                    else:
                        pending = fin

        # ---------------- Phase 3: out proj + residual + LayerNorm ----------------
        with ExitStack() as p3:
            stp = p3.enter_context(tc.tile_pool(name="stats", bufs=12))
            np_ = p3.enter_context(tc.tile_pool(name="norm", bufs=6))
            outp = p3.enter_context(tc.tile_pool(name="outp", bufs=6))
            xqp = p3.enter_context(tc.tile_pool(name="xqp", bufs=4))
            ops = p3.enter_context(tc.tile_pool(name="ps3", bufs=8, space="PSUM"))

            for sc in range(SQ // P):
                xqt = xqp.tile([P, D], f16, tag="xq")
                nc.sync.dma_start(xqt[:], xqbo[sc * P:(sc + 1) * P, :])
                pse = [ops.tile([P, NT], f32, tag="ps", name=f"ps3_{et}")
                       for et in range(2)]
                for c in range(C4):
                    for et in range(2):
                        nc.tensor.matmul(
                            pse[et][:], lhsT=ctxt8[:, c, :, sc * P:(sc + 1) * P],
                            rhs=wo_sb[:, c, :, et * NT:(et + 1) * NT],
                            start=(c == 0), stop=False, perf_mode=DR)
                # += 128*(x + bo) via scaled identity; PSUM = 128*y, and
                # LayerNorm is scale-invariant.
                for et in range(2):
                    nc.tensor.matmul(pse[et][:], lhsT=id_sb[:],
                                     rhs=xqt[:, et * NT:(et + 1) * NT],
                                     start=False, stop=True)
                bst = stp.tile([P, 2, 6], f32, tag="bst")
                for et in range(2):
                    nc.vector.bn_stats(bst[:, et, :], pse[et][:])
                mv = stp.tile([P, 2], f32, tag="mv")
                nc.vector.bn_aggr(mv[:], bst[:])
                std = stp.tile([P, 1], f32, tag="std")
                nc.scalar.activation(std[:], mv[:, 1:2], AF.Sqrt, bias=eps_t[:])
                rstd = stp.tile([P, 1], f32, tag="rstd")
                nc.vector.reciprocal(rstd[:], std[:])
                nmr = stp.tile([P, 1], f32, tag="nmr")
                nc.vector.scalar_tensor_tensor(nmr[:], in0=mv[:, 0:1], scalar=-1.0,
                                               in1=rstd[:], op0=OP.mult, op1=OP.mult)
                for et in range(2):
                    if t["gb_trivial"]:
                        # gamma==1, beta==0 (checked on the host): the
                        # normalize activation writes the final output —
                        # drops two engine hops from the drain chain.
                        o = outp.tile([P, NT], f32, tag="o")
                        nc.scalar.activation(o[:], pse[et][:], AF.Identity,
                                             bias=nmr[:], scale=rstd[:])
                    else:
                        tn = np_.tile([P, NT], f32, tag="tn")
                        nc.scalar.activation(tn[:], pse[et][:], AF.Identity,
                                             bias=nmr[:], scale=rstd[:])
                        o = outp.tile([P, NT], f32, tag="o")
                        nc.vector.tensor_tensor(o[:], tn[:],
                                                cbc[:, D + et * NT:D + (et + 1) * NT],
                                                OP.mult)
                        nc.gpsimd.tensor_tensor(o[:], o[:],
                                                cbc[:, 2 * D + et * NT:2 * D + (et + 1) * NT],
                                                OP.add)
                    nc.sync.dma_start(out[sc * P:(sc + 1) * P, et * NT:(et + 1) * NT],
                                      o[:])


def _build(gb_trivial):
    key = ("nc", gb_trivial)
    if key in _CACHE:
        return _CACHE[key]
    from concourse import bacc, mybir
    import concourse.tile as tile

    f32 = mybir.dt.float32
    f16 = mybir.dt.float16
    f8 = mybir.dt.float8e4
    nc = bacc.Bacc("TRN2", target_bir_lowering=False, debug=False)
    t = {}
    t["xt8"] = nc.dram_tensor("xt8", [D, T], f8, kind="ExternalInput")
    t["xtq8"] = nc.dram_tensor("xtq8", [D, SQ], f8, kind="ExternalInput")
    t["wq8"] = nc.dram_tensor("wq8", [D, D], f8, kind="ExternalInput")
    t["wk8"] = nc.dram_tensor("wk8", [D, D], f8, kind="ExternalInput")
    t["wv8"] = nc.dram_tensor("wv8", [D, D], f8, kind="ExternalInput")
    t["wo8"] = nc.dram_tensor("wo8", [D, D], f8, kind="ExternalInput")
    t["xqbo"] = nc.dram_tensor("xqbo", [SQ, D], f16, kind="ExternalInput")
    t["bq8"] = nc.dram_tensor("bq8", [D], f32, kind="ExternalInput")
    t["bk"] = nc.dram_tensor("bk", [D], f32, kind="ExternalInput")
    t["cgb"] = nc.dram_tensor("cgb", [3 * D], f32, kind="ExternalInput")
    t["ident"] = nc.dram_tensor("ident", [P, P], f16, kind="ExternalInput")
    t["out"] = nc.dram_tensor("out", [SQ, D], f32, kind="ExternalOutput")

    t["gb_trivial"] = gb_trivial
    with tile.TileContext(nc) as tc:
        _emit(tc, t)
    nc.compile()
    _CACHE[key] = nc
    return nc


def _chunk_fp8(a):
    """[1024, N] -> fp8 with rows pre-arranged as (c, j, p); layout is
    identity because d = c*256 + j*128 + p matches C-order reshape."""
    import ml_dtypes
    return np.clip(a, -240.0, 240.0).astype(ml_dtypes.float8_e4m3)


def _prep_inputs(x, Wq, bq, Wk, bk, Wv, bv, Wo, bo, ln_gamma, ln_beta):
    """Host-side sharding/layout prep. Returns per-core input maps."""
    f = np.float32
    x = np.asarray(x, f)
    wq8 = _chunk_fp8(np.asarray(Wq, f).T * 8.0)
    wk8 = _chunk_fp8(np.asarray(Wk, f).T * 8.0)
    wv8 = _chunk_fp8(np.asarray(Wv, f).T * 8.0)
    wo8 = _chunk_fp8(np.asarray(Wo, f).T * 8.0)
    bq8 = np.asarray(bq, f) / 8.0
    cgb = np.concatenate([np.asarray(bv, f), np.asarray(ln_gamma, f),
                          np.asarray(ln_beta, f)])
    ident = (128.0 * np.eye(P, dtype=f)).astype(np.float16)
    bo_f = np.asarray(bo, f)
    in_maps = []
    for c in range(N_CORES):
        b, half = c // 2, c % 2
        xb = x[b]                                        # [2048, 1024]
        xslice = xb[half * SQ:(half + 1) * SQ]           # [1024, 1024]
        in_maps.append({
            "xt8": _chunk_fp8(np.ascontiguousarray(xb.T)),
            "xtq8": _chunk_fp8(np.ascontiguousarray(xslice.T)),
            "wq8": wq8, "wk8": wk8, "wv8": wv8, "wo8": wo8,
            "xqbo": (xslice + bo_f).astype(np.float16),
            "bq8": bq8, "bk": np.asarray(bk, f),
            "cgb": cgb, "ident": ident,
        })
    return in_maps


def _ensure_axon_hooks_shim():
    """This image's `antenv` lacks the `axon_hooks` registry module that
    `run_bass_kernel_spmd(trace=True)` imports. Provide it (hook installed
    from the boot .so when available, else None -> tracing degrades
    gracefully instead of raising ImportError)."""
    import importlib
    import types

    try:
        importlib.import_module("antenv.axon_hooks")
        return
    except ImportError:
        pass
    mod = types.ModuleType("antenv.axon_hooks")
    _state = {"hook": None}
    mod.set_axon_ntff_profile_hook = lambda h: _state.update(hook=h)
    mod.get_axon_ntff_profile_hook = lambda: _state["hook"]
    sys.modules["antenv.axon_hooks"] = mod
    try:
        import antenv
        antenv.axon_hooks = mod
    except Exception:
        pass
    try:
        from trn_agent_boot.trn_boot import _ntff_profile_via_ctypes
        so = "/opt/axon/libaxon_pjrt.so"
        if os.path.exists(so):
            mod.set_axon_ntff_profile_hook(_ntff_profile_via_ctypes(so))
    except Exception:
        pass


def kernel(**inputs):
    global LAST_RESULTS
    _ensure_axon_hooks_shim()
    from concourse.bass_utils import run_bass_kernel_spmd

    gb_trivial = (np.allclose(np.asarray(inputs["ln_gamma"]), 1.0)
                  and np.allclose(np.asarray(inputs["ln_beta"]), 0.0))
    nc = _build(gb_trivial)
    in_maps = _prep_inputs(**inputs)
    trace = bool(os.environ.get("MHA_TRACE"))
    res = run_bass_kernel_spmd(nc, in_maps, core_ids=list(range(N_CORES)),
                               trace=trace)
    LAST_RESULTS = res
    out = np.empty((B, S, D), np.float32)
    for c in range(N_CORES):
        b, half = c // 2, c % 2
        out[b, half * SQ:(half + 1) * SQ, :] = res.results[c]["out"]
    return out


if __name__ == "__main__":
    from reference import setup_inputs, reference
    import jax
    with jax.default_device(jax.devices("cpu")[0]):
        inp = {k: np.asarray(v) for k, v in setup_inputs().items()}
        exp = np.asarray(reference(**inp))
    act = kernel(**inp)
    err = np.linalg.norm(act - exp) / np.linalg.norm(exp)
    print("Relative error:", err)
